# revision 4
# baseline (speedup 1.0000x reference)
"""Causal self-attention (B=4, T=2048, C=1024, H=16) on 8 TRN2 NeuronCores.

Sharding: 8 cores = 4 batches x 2 head-groups (Megatron tensor-parallel over
heads + data-parallel over batch). Each core computes, for its batch b and its
8 heads, a fused flash-style pipeline over 512-token tiles t:

  phase t: attention for query tile t (all 4 head pairs, causal key tiles)
           interleaved by the Tile scheduler with stage-1 qkv projection of
           tile t+1 and stage-3 output projection of tile t, so the PE
           fills the gaps where attention waits on the Scalar-engine exp.

Layouts (PSUM accumulation fp32):
  q8/k8 [fp8e4m3; head h on partitions 32g..32g+31 (g=h%4), dim planes
    (0-31, 32-63) adjacent on the free dim] so the QK^T logits run as
    fp8 DoubleRow matmuls (2 contraction planes, 0.5 cycles/column —
    half the PE time of the bf16 pair path). exp(x*0.125) on the Scalar
    engine applies the 1/sqrt(hd) scale for free.
  V [tokens on partitions, head-major dims on free, bf16] with an appended
    ones-column per head so the PV matmul computes softmax sums for free.
  exp on the Scalar engine straight out of PSUM (no max-subtraction: logits
  are O(1) by construction); the causal triangle of diagonal blocks is
  zeroed by a 0/1 bf16 mask multiply on the Vector engine after the exp.
  A short burst of dependency-free warm-up matmuls at t=0 ramps the PE
  p-state while the first x/w DMAs land.

Host: shards inputs (bf16), sums the two partial outputs per batch, adds b_proj.
"""

import sys

if "/opt/trn_rl_repo" not in sys.path:
    sys.path.insert(0, "/opt/trn_rl_repo")

from contextlib import ExitStack

import numpy as np
import ml_dtypes

import concourse.bass as bass
import concourse.tile as tile
from concourse import bacc, mybir
from concourse.bass_utils import run_bass_kernel_spmd

F32 = mybir.dt.float32
BF16 = mybir.dt.bfloat16
FP8 = mybir.dt.float8e4
AF = mybir.ActivationFunctionType
ADD = mybir.AluOpType.add
DR = mybir.MatmulPerfMode.DoubleRow

B, T, C = 4, 2048, 1024
H, HD = 16, 64
NHL = 8          # heads per core (local)
NPAIR = 4        # head pairs per core
P = 128
TQ = 512         # query tile (free dim)
TJ = 128         # key tile (partitions)
NIT = T // TQ    # 4 query tiles
NTS = T // P     # 16 token sub-tiles
NCT = C // P     # 8 contraction tiles over C
N_WARM = 44      # dep-free PE warm-up matmuls (64 cols each)


def build_kernel():
    nc = bacc.Bacc("TRN2", target_bir_lowering=False)

    xt = nc.declare_dram_parameter("xt", [C, T], BF16, isOutput=False)
    wqk = nc.declare_dram_parameter("wqk", [P, NCT, 1024], BF16, isOutput=False)
    bqk = nc.declare_dram_parameter("bqk", [P, 8], F32, isOutput=False)
    wv = nc.declare_dram_parameter("wv", [P, NCT, 512], BF16, isOutput=False)
    bv = nc.declare_dram_parameter("bv", [1, 512], F32, isOutput=False)
    wp = nc.declare_dram_parameter("wp", [P, NPAIR, 1024], BF16, isOutput=False)
    tri = nc.declare_dram_parameter("tri", [P, 2, P], BF16, isOutput=False)
    out = nc.declare_dram_parameter("out", [T, C], F32, isOutput=True)

    with tile.TileContext(nc) as tc, ExitStack() as ctx:
        persist = ctx.enter_context(tc.tile_pool(name="persist", bufs=1))
        xp = ctx.enter_context(tc.tile_pool(name="xp", bufs=4))
        attp = ctx.enter_context(tc.tile_pool(name="attp", bufs=8))
        rtp = ctx.enter_context(tc.tile_pool(name="rtp", bufs=6))
        rbp = ctx.enter_context(tc.tile_pool(name="rbp", bufs=6))
        otp = ctx.enter_context(tc.tile_pool(name="otp", bufs=3))
        accps = ctx.enter_context(tc.tile_pool(name="accps", bufs=2, space="PSUM"))
        qkps = ctx.enter_context(tc.tile_pool(name="qkps", bufs=2, space="PSUM"))
        pvps = ctx.enter_context(tc.tile_pool(name="pvps", bufs=2, space="PSUM"))

        # q8/k8: [part(32g+dim%32), hg, dim-plane, tok]; head h = 4*hg + g
        q8_sb = persist.tile([P, 2, 2, T], FP8)
        k8_sb = persist.tile([P, 2, 2, T], FP8)
        v_sb = persist.tile([P, NTS, NHL, HD + 1], BF16)
        y_sb = persist.tile([P, NPAIR, T], BF16)
        wqk_sb = persist.tile([P, NCT, 1024], BF16)
        wv_sb = persist.tile([P, NCT, 512], BF16)
        wp_sb = persist.tile([P, NPAIR, 1024], BF16)
        bqk_sb = persist.tile([P, 8], F32)
        bv_sb = persist.tile([P, 512], F32)
        tri_sb = persist.tile([P, 2, P], BF16)
        warm_sb = persist.tile([P, 64], BF16)

        def s1_load(t, split=False):
            # one DMA per 512-token x tile: [C_chunk on partitions, chunk, tok]
            t0 = t * TQ
            xi = xp.tile([P, NCT, TQ], BF16, tag="xc")
            src = xt[:, t0 : t0 + TQ].rearrange("(c p) t -> p c t", p=P)
            if split:
                # two DMAs so the first c-chunks land sooner at kernel start
                nc.sync.dma_start(xi[:, 0:4, :], src[:, 0:4, :])
                nc.sync.dma_start(xi[:, 4:8, :], src[:, 4:8, :])
            else:
                nc.sync.dma_start(xi, src)
            return xi

        def s1_compute(t, xi):
            t0 = t * TQ
            # q (m 0-3) and k (m 4-7) blocks: out [f-part, t-free]
            for m in range(8):
                ps = accps.tile([P, TQ], F32, tag="acc")
                for c in range(NCT):
                    nc.tensor.matmul(
                        ps,
                        wqk_sb[:, c, m * P : (m + 1) * P],
                        xi[:, c, :],
                        start=(c == 0),
                        stop=(c == NCT - 1),
                    )
                dst = q8_sb if m < 4 else k8_sb
                mm = m % 4
                nc.vector.tensor_scalar_add(
                    dst[:, mm // 2, mm % 2, t0 : t0 + TQ], ps, bqk_sb[:, m : m + 1]
                )
            # v blocks: out [t-part, f-free(head-major)]
            for s in range(TQ // P):
                ps = accps.tile([P, 512], F32, tag="acc")
                for c in range(NCT):
                    nc.tensor.matmul(
                        ps,
                        xi[:, c, s * P : (s + 1) * P],
                        wv_sb[:, c, :],
                        start=(c == 0),
                        stop=(c == NCT - 1),
                    )
                tsub = t * (TQ // P) + s
                nc.vector.tensor_tensor(
                    v_sb[:, tsub, :, 0:HD],
                    ps.rearrange("p (h d) -> p h d", h=NHL),
                    bv_hd,
                    ADD,
                )

        def att_phase(it):
            i0 = it * TQ
            njt = (i0 + TQ) // TJ
            for a in range(NPAIR):
                hg = a // 2
                pv0 = pvps.tile([P, TQ], F32, tag="pv", name=f"pv0_{a}_{it}")
                pv1 = pvps.tile([P, TQ], F32, tag="pv", name=f"pv1_{a}_{it}")
                for jt in range(njt):
                    j0 = jt * TJ
                    d = j0 - i0
                    istart = max(d, 0)
                    nn = TQ - istart
                    qk = qkps.tile([P, 2, TQ], F32, tag="qk")
                    for e in (0, 1):
                        bp = 32 * (2 * (a % 2) + e)
                        nc.tensor.matmul(
                            qk[:, e, istart:TQ],
                            k8_sb[bp : bp + 32, hg, :, j0 : j0 + TJ],
                            q8_sb[bp : bp + 32, hg, :, i0 + istart : i0 + TQ],
                            start=True,
                            stop=True,
                            perf_mode=DR,
                            tile_position=(bp, 0),
                        )
                    att = attp.tile([P, 2, TQ], BF16, tag="att")
                    nc.scalar.activation(
                        att[:, :, 0:nn], qk[:, :, istart:TQ], AF.Exp, scale=0.125
                    )
                    if d >= 0:
                        # zero the upper triangle of the diagonal block (the
                        # first TJ query columns) off the PE's critical path
                        nc.vector.tensor_tensor(
                            att[:, :, 0:TJ], att[:, :, 0:TJ], tri_sb,
                            mybir.AluOpType.mult,
                        )
                    last = jt == njt - 1
                    nc.tensor.matmul(
                        pv0[0 : HD + 1, istart:TQ],
                        v_sb[:, jt, 2 * a, :],
                        att[:, 0, 0:nn],
                        start=(jt == 0),
                        stop=last,
                    )
                    nc.tensor.matmul(
                        pv1[0 : HD + 1, istart:TQ],
                        v_sb[:, jt, 2 * a + 1, :],
                        att[:, 1, 0:nn],
                        start=(jt == 0),
                        stop=last,
                    )
                # normalize: softmax sums sit at row HD of each pv tile
                rt = rtp.tile([P, TQ], F32, tag="rt")
                rb = rbp.tile([P, TQ], F32, tag="rb")
                nc.vector.reciprocal(rt[HD : HD + 1, :], pv0[HD : HD + 1, :])
                nc.sync.dma_start(rb[0:1, :], rt[HD : HD + 1, :])
                nc.gpsimd.partition_broadcast(rb[0:HD, :], rb[0:1, :])
                nc.vector.tensor_mul(
                    y_sb[0:HD, a, i0 : i0 + TQ], pv0[0:HD, :], rb[0:HD, :]
                )
                rt1 = rtp.tile([P, TQ], F32, tag="rt")
                rb1 = rbp.tile([P, TQ], F32, tag="rb")
                nc.vector.reciprocal(rt1[HD : HD + 1, :], pv1[HD : HD + 1, :])
                nc.sync.dma_start(rb1[0:1, :], rt1[HD : HD + 1, :])
                nc.gpsimd.partition_broadcast(rb1[0:HD, :], rb1[0:1, :])
                yt = rtp.tile([P, TQ], BF16, tag="yt")
                nc.vector.tensor_mul(yt[0:HD, :], pv1[0:HD, :], rb1[0:HD, :])
                nc.sync.dma_start(y_sb[HD:P, a, i0 : i0 + TQ], yt[0:HD, :])

        def s3_tile(tt, fine=True):
            ot_sb = otp.tile([P, 1024], F32, tag="osb")
            for ot in range(2):
                ps = accps.tile([P, 512], F32, tag="acc")
                for a in range(NPAIR):
                    nc.tensor.matmul(
                        ps,
                        y_sb[:, a, tt * P : (tt + 1) * P],
                        wp_sb[:, a, ot * 512 : (ot + 1) * 512],
                        start=(a == 0),
                        stop=(a == NPAIR - 1),
                    )
                nc.vector.tensor_copy(ot_sb[:, ot * 512 : (ot + 1) * 512], ps)
                if fine:
                    # ship each half as soon as its copy lands
                    nc.sync.dma_start(
                        out[tt * P : (tt + 1) * P, ot * 512 : (ot + 1) * 512],
                        ot_sb[:, ot * 512 : (ot + 1) * 512],
                    )
            if not fine:
                nc.sync.dma_start(out[tt * P : (tt + 1) * P, :], ot_sb)

        # ---------------- fused pipeline ----------------
        # PE warm-up: dependency-free matmuls start the p-state ramp at t~0
        # and keep the PE busy while the first x/wqk DMAs land.
        nc.vector.memset(warm_sb, 1.0)
        for _ in range(N_WARM):
            wps = accps.tile([P, TQ], F32, tag="acc")
            nc.tensor.matmul(
                wps[0:64, 0:64], warm_sb[:, 0:64], warm_sb[:, 0:64],
                start=True, stop=True,
            )
        # head: x(0) + weights interleaved finely — the DMA device transfers
        # in issue order, and the first m-chunk accumulations need the low-c
        # x/wqk chunks first
        x0 = xp.tile([P, NCT, TQ], BF16, tag="xc")
        src0 = xt[:, 0:TQ].rearrange("(c p) t -> p c t", p=P)
        nc.sync.dma_start(x0[:, 0:4, :], src0[:, 0:4, :])
        nc.sync.dma_start(wqk_sb[:, 0:2, :], wqk[:, 0:2, :])
        nc.sync.dma_start(wqk_sb[:, 2:4, :], wqk[:, 2:4, :])
        nc.sync.dma_start(x0[:, 4:8, :], src0[:, 4:8, :])
        nc.sync.dma_start(wqk_sb[:, 4:6, :], wqk[:, 4:6, :])
        nc.sync.dma_start(wqk_sb[:, 6:8, :], wqk[:, 6:8, :])
        nc.sync.dma_start(wv_sb, wv[:])
        nc.sync.dma_start(bqk_sb, bqk[:])
        nc.sync.dma_start(tri_sb, tri[:])
        nc.sync.dma_start(bv_sb[0:1, :], bv[:])
        nc.gpsimd.partition_broadcast(bv_sb[:, :], bv_sb[0:1, :])
        bv_hd = bv_sb.rearrange("p (h d) -> p h d", h=NHL)
        # ones columns of the augmented V
        nc.vector.memset(v_sb[:, :, :, HD : HD + 1], 1.0)

        s1_compute(0, x0)
        nc.sync.dma_start(wp_sb, wp[:])
        xs = {}
        for t in range(NIT):
            if t + 1 < NIT:
                xs[t + 1] = s1_load(t + 1)
            att_phase(t)
            if t + 1 < NIT:
                s1_compute(t + 1, xs[t + 1])
            # stage 3 for this query tile's tokens: back-fills the PE while
            # the next attention phase waits on the Scalar-engine exp
            for tt in range(4 * t, 4 * t + 4):
                s3_tile(tt)

    nc.compile()
    return nc


_NC_CACHE = None


def _get_nc():
    global _NC_CACHE
    if _NC_CACHE is None:
        _NC_CACHE = build_kernel()
    return _NC_CACHE


def _shard_inputs(x, w_qkv, b_qkv, w_proj):
    """Build the 8 per-core input maps. Core id = 2*batch + head_group."""
    bf = ml_dtypes.bfloat16
    tri01 = np.where(
        np.arange(P)[None, :] >= np.arange(P)[:, None], 1.0, 0.0
    )
    tri_np = np.ascontiguousarray(
        np.stack([tri01, tri01], axis=1)
    ).astype(bf)  # [P, 2, P]

    # q/k m-block row permutation: m in 0..7 -> (q|k, hg=(m%4)//2, pl=m%2);
    # partition p = 32*gg + i  ->  local head 4*hg + gg, dim 32*pl + i
    perm = np.empty((8, P), np.int64)
    for m in range(8):
        base = 0 if m < 4 else 1024
        mm = m % 4
        hg, pl = mm // 2, mm % 2
        for gg in range(4):
            h = 4 * hg + gg
            perm[m, 32 * gg : 32 * gg + 32] = (
                base + 64 * h + 32 * pl + np.arange(32)
            )

    in_maps = []
    for b in range(B):
        xt = np.ascontiguousarray(x[b].T).astype(bf)  # [C, T]
        for g in range(2):
            s = slice(g * 512, (g + 1) * 512)
            goff = g * 512  # global row offset of this head group's q rows
            rows = perm + goff  # k rows offset by 1024 already in perm base
            wqk_full = w_qkv[rows.reshape(-1)]  # [1024 f, 1024 c]
            wqk_arr = np.ascontiguousarray(
                wqk_full.T.reshape(NCT, P, 1024).transpose(1, 0, 2)
            ).astype(bf)
            bqk_full = b_qkv[rows.reshape(-1)]
            bqk_arr = np.ascontiguousarray(bqk_full.reshape(8, P).T)
            wv_rows = w_qkv[2048:3072][s]  # [512 f, 1024 c]
            wv_arr = np.ascontiguousarray(
                wv_rows.T.reshape(NCT, P, 512).transpose(1, 0, 2)
            ).astype(bf)
            bv_arr = np.ascontiguousarray(b_qkv[2048:3072][s][None, :])
            wp_rhs = w_proj[:, s].T  # [512 hd, 1024 o]
            wp_arr = np.ascontiguousarray(
                wp_rhs.reshape(NPAIR, P, 1024).transpose(1, 0, 2)
            ).astype(bf)
            in_maps.append(
                {
                    "xt": xt,
                    "wqk": wqk_arr,
                    "bqk": bqk_arr.astype(np.float32),
                    "wv": wv_arr,
                    "bv": bv_arr.astype(np.float32),
                    "wp": wp_arr,
                    "tri": tri_np,
                }
            )
    return in_maps


def kernel(x, w_qkv, b_qkv, w_proj, b_proj, _trace=False, _trace_kwargs=None):
    x = np.asarray(x, dtype=np.float32)
    w_qkv = np.asarray(w_qkv, dtype=np.float32)
    b_qkv = np.asarray(b_qkv, dtype=np.float32)
    w_proj = np.asarray(w_proj, dtype=np.float32)
    b_proj = np.asarray(b_proj, dtype=np.float32)

    nc = _get_nc()
    in_maps = _shard_inputs(x, w_qkv, b_qkv, w_proj)
    res = run_bass_kernel_spmd(
        nc, in_maps, core_ids=list(range(8)), trace=_trace,
        **(_trace_kwargs or {}),
    )
    out = np.empty((B, T, C), np.float32)
    for b in range(B):
        out[b] = res.results[2 * b]["out"] + res.results[2 * b + 1]["out"] + b_proj
    if _trace:
        return out, res
    return out


# revision 5
# speedup vs baseline: 1.0702x; 1.0702x over previous
"""Causal self-attention (B=4, T=2048, C=1024, H=16) on 8 TRN2 NeuronCores.

Sharding: 8 cores = 4 batches x 2 head-groups (Megatron tensor-parallel over
heads + data-parallel over batch). Each core computes, for its batch b and its
8 heads, a fused flash-style pipeline over 512-token tiles t:

  phase t: attention for query tile t (all 4 head pairs, causal key tiles)
           interleaved by the Tile scheduler with stage-1 qkv projection of
           tile t+1 and stage-3 output projection of tile t, so the PE
           fills the gaps where attention waits on the Scalar-engine exp.

Layouts (PSUM accumulation fp32):
  q8/k8 [fp8e4m3; head h on partitions 32g..32g+31 (g=h%4), dim planes
    (0-31, 32-63) adjacent on the free dim] so the QK^T logits run as
    fp8 DoubleRow matmuls (2 contraction planes, 0.5 cycles/column —
    half the PE time of the bf16 pair path). exp(x*0.125) on the Scalar
    engine applies the 1/sqrt(hd) scale for free.
  V [tokens on partitions, head-major dims on free, bf16] with an appended
    ones-column per head so the PV matmul computes softmax sums for free.
  exp on the Scalar engine straight out of PSUM (no max-subtraction: logits
  are O(1) by construction); the causal triangle of diagonal blocks is
  zeroed by a 0/1 bf16 mask multiply on the Vector engine after the exp.
  A short burst of dependency-free warm-up matmuls at t=0 ramps the PE
  p-state while the first x/w DMAs land.

Host: shards inputs (bf16), sums the two partial outputs per batch, adds b_proj.
"""

import sys

if "/opt/trn_rl_repo" not in sys.path:
    sys.path.insert(0, "/opt/trn_rl_repo")

from contextlib import ExitStack

import numpy as np
import ml_dtypes

import concourse.bass as bass
import concourse.tile as tile
from concourse import bacc, mybir
from concourse.bass_utils import run_bass_kernel_spmd

F32 = mybir.dt.float32
BF16 = mybir.dt.bfloat16
FP8 = mybir.dt.float8e4
AF = mybir.ActivationFunctionType
ADD = mybir.AluOpType.add
DR = mybir.MatmulPerfMode.DoubleRow

B, T, C = 4, 2048, 1024
H, HD = 16, 64
NHL = 8          # heads per core (local)
NPAIR = 4        # head pairs per core
P = 128
TQ = 512         # query tile (free dim)
TJ = 128         # key tile (partitions)
NIT = T // TQ    # 4 query tiles
NTS = T // P     # 16 token sub-tiles
NCT = C // P     # 8 contraction tiles over C
N_WARM = 44      # dep-free PE warm-up matmuls (64 cols each)


def build_kernel():
    nc = bacc.Bacc("TRN2", target_bir_lowering=False)

    xt = nc.declare_dram_parameter("xt", [C, T], BF16, isOutput=False)
    wqk = nc.declare_dram_parameter("wqk", [P, NCT, 1024], BF16, isOutput=False)
    bqk = nc.declare_dram_parameter("bqk", [P, 8], F32, isOutput=False)
    wv = nc.declare_dram_parameter("wv", [P, NCT, 512], BF16, isOutput=False)
    bv = nc.declare_dram_parameter("bv", [1, 512], F32, isOutput=False)
    wp = nc.declare_dram_parameter("wp", [P, NPAIR, 1024], BF16, isOutput=False)
    tri = nc.declare_dram_parameter("tri", [P, 2, P], BF16, isOutput=False)
    out = nc.declare_dram_parameter("out", [T, C], F32, isOutput=True)

    with tile.TileContext(nc) as tc, ExitStack() as ctx:
        persist = ctx.enter_context(tc.tile_pool(name="persist", bufs=1))
        xp = ctx.enter_context(tc.tile_pool(name="xp", bufs=4))
        attp = ctx.enter_context(tc.tile_pool(name="attp", bufs=8))
        rtp = ctx.enter_context(tc.tile_pool(name="rtp", bufs=6))
        rbp = ctx.enter_context(tc.tile_pool(name="rbp", bufs=6))
        otp = ctx.enter_context(tc.tile_pool(name="otp", bufs=3))
        accps = ctx.enter_context(tc.tile_pool(name="accps", bufs=2, space="PSUM"))
        qkps = ctx.enter_context(tc.tile_pool(name="qkps", bufs=2, space="PSUM"))
        pvps = ctx.enter_context(tc.tile_pool(name="pvps", bufs=2, space="PSUM"))

        # q8/k8: [part(32g+dim%32), hg, dim-plane, tok]; head h = 4*hg + g
        q8_sb = persist.tile([P, 2, 2, T], FP8)
        k8_sb = persist.tile([P, 2, 2, T], FP8)
        v_sb = persist.tile([P, NTS, NHL, HD + 1], BF16)
        y_sb = persist.tile([P, NPAIR, T], BF16)
        wqk_sb = persist.tile([P, NCT, 1024], BF16)
        wv_sb = persist.tile([P, NCT, 512], BF16)
        wp_sb = persist.tile([P, NPAIR, 1024], BF16)
        bqk_sb = persist.tile([P, 8], F32)
        bv_sb = persist.tile([P, 512], F32)
        tri_sb = persist.tile([P, 2, P], BF16)
        warm_sb = persist.tile([P, 64], BF16)

        def s1_load(t, split=False):
            # one DMA per 512-token x tile: [C_chunk on partitions, chunk, tok]
            t0 = t * TQ
            xi = xp.tile([P, NCT, TQ], BF16, tag="xc")
            src = xt[:, t0 : t0 + TQ].rearrange("(c p) t -> p c t", p=P)
            if split:
                # two DMAs so the first c-chunks land sooner at kernel start
                nc.sync.dma_start(xi[:, 0:4, :], src[:, 0:4, :])
                nc.sync.dma_start(xi[:, 4:8, :], src[:, 4:8, :])
            else:
                nc.sync.dma_start(xi, src)
            return xi

        def s1_compute(t, xi):
            t0 = t * TQ
            # q (m 0-3) and k (m 4-7) blocks: out [f-part, t-free]
            for m in range(8):
                ps = accps.tile([P, TQ], F32, tag="acc")
                for c in range(NCT):
                    nc.tensor.matmul(
                        ps,
                        wqk_sb[:, c, m * P : (m + 1) * P],
                        xi[:, c, :],
                        start=(c == 0),
                        stop=(c == NCT - 1),
                    )
                dst = q8_sb if m < 4 else k8_sb
                mm = m % 4
                nc.vector.tensor_scalar_add(
                    dst[:, mm // 2, mm % 2, t0 : t0 + TQ], ps, bqk_sb[:, m : m + 1]
                )
            # v blocks: out [t-part, f-free(head-major)]
            for s in range(TQ // P):
                ps = accps.tile([P, 512], F32, tag="acc")
                for c in range(NCT):
                    nc.tensor.matmul(
                        ps,
                        xi[:, c, s * P : (s + 1) * P],
                        wv_sb[:, c, :],
                        start=(c == 0),
                        stop=(c == NCT - 1),
                    )
                tsub = t * (TQ // P) + s
                nc.vector.tensor_tensor(
                    v_sb[:, tsub, :, 0:HD],
                    ps.rearrange("p (h d) -> p h d", h=NHL),
                    bv_hd,
                    ADD,
                )

        def att_phase(it):
            i0 = it * TQ
            njt = (i0 + TQ) // TJ
            for a in range(NPAIR):
                hg = a // 2
                pv0 = pvps.tile([P, TQ], F32, tag="pv", name=f"pv0_{a}_{it}")
                pv1 = pvps.tile([P, TQ], F32, tag="pv", name=f"pv1_{a}_{it}")
                for jt in range(njt):
                    j0 = jt * TJ
                    d = j0 - i0
                    istart = max(d, 0)
                    nn = TQ - istart
                    qk = qkps.tile([P, 2, TQ], F32, tag="qk")
                    for e in (0, 1):
                        bp = 32 * (2 * (a % 2) + e)
                        nc.tensor.matmul(
                            qk[:, e, istart:TQ],
                            k8_sb[bp : bp + 32, hg, :, j0 : j0 + TJ],
                            q8_sb[bp : bp + 32, hg, :, i0 + istart : i0 + TQ],
                            start=True,
                            stop=True,
                            perf_mode=DR,
                            tile_position=(bp, 0),
                        )
                    att = attp.tile([P, 2, TQ], BF16, tag="att")
                    nc.scalar.activation(
                        att[:, :, 0:nn], qk[:, :, istart:TQ], AF.Exp, scale=0.125
                    )
                    if d >= 0:
                        # zero the upper triangle of the diagonal block (the
                        # first TJ query columns) off the PE's critical path
                        nc.vector.tensor_tensor(
                            att[:, :, 0:TJ], att[:, :, 0:TJ], tri_sb,
                            mybir.AluOpType.mult,
                        )
                    last = jt == njt - 1
                    nc.tensor.matmul(
                        pv0[0 : HD + 1, istart:TQ],
                        v_sb[:, jt, 2 * a, :],
                        att[:, 0, 0:nn],
                        start=(jt == 0),
                        stop=last,
                    )
                    nc.tensor.matmul(
                        pv1[0 : HD + 1, istart:TQ],
                        v_sb[:, jt, 2 * a + 1, :],
                        att[:, 1, 0:nn],
                        start=(jt == 0),
                        stop=last,
                    )
                # normalize: softmax sums sit at row HD of each pv tile
                rt = rtp.tile([P, TQ], F32, tag="rt")
                rb = rbp.tile([P, TQ], F32, tag="rb")
                nc.vector.reciprocal(rt[HD : HD + 1, :], pv0[HD : HD + 1, :])
                nc.sync.dma_start(rb[0:1, :], rt[HD : HD + 1, :])
                nc.gpsimd.partition_broadcast(rb[0:HD, :], rb[0:1, :])
                nc.vector.tensor_mul(
                    y_sb[0:HD, a, i0 : i0 + TQ], pv0[0:HD, :], rb[0:HD, :]
                )
                rt1 = rtp.tile([P, TQ], F32, tag="rt")
                rb1 = rbp.tile([P, TQ], F32, tag="rb")
                nc.vector.reciprocal(rt1[HD : HD + 1, :], pv1[HD : HD + 1, :])
                nc.sync.dma_start(rb1[0:1, :], rt1[HD : HD + 1, :])
                nc.gpsimd.partition_broadcast(rb1[0:HD, :], rb1[0:1, :])
                yt = rtp.tile([P, TQ], BF16, tag="yt")
                nc.vector.tensor_mul(yt[0:HD, :], pv1[0:HD, :], rb1[0:HD, :])
                nc.sync.dma_start(y_sb[HD:P, a, i0 : i0 + TQ], yt[0:HD, :])

        def s3_tile(tt, fine=True):
            ot_sb = otp.tile([P, 1024], F32, tag="osb")
            for ot in range(2):
                ps = accps.tile([P, 512], F32, tag="acc")
                for a in range(NPAIR):
                    nc.tensor.matmul(
                        ps,
                        y_sb[:, a, tt * P : (tt + 1) * P],
                        wp_sb[:, a, ot * 512 : (ot + 1) * 512],
                        start=(a == 0),
                        stop=(a == NPAIR - 1),
                    )
                nc.vector.tensor_copy(ot_sb[:, ot * 512 : (ot + 1) * 512], ps)
                if fine:
                    # ship each half as soon as its copy lands
                    nc.sync.dma_start(
                        out[tt * P : (tt + 1) * P, ot * 512 : (ot + 1) * 512],
                        ot_sb[:, ot * 512 : (ot + 1) * 512],
                    )
            if not fine:
                nc.sync.dma_start(out[tt * P : (tt + 1) * P, :], ot_sb)

        # ---------------- fused pipeline ----------------
        # PE warm-up: dependency-free matmuls start the p-state ramp at t~0
        # and keep the PE busy while the first x/wqk DMAs land.
        nc.vector.memset(warm_sb, 1.0)
        for _ in range(N_WARM):
            wps = accps.tile([P, TQ], F32, tag="acc")
            nc.tensor.matmul(
                wps[0:64, 0:64], warm_sb[:, 0:64], warm_sb[:, 0:64],
                start=True, stop=True,
            )
        # head: x(0) + weights interleaved finely — the DMA device transfers
        # in issue order, and the first m-chunk accumulations need the low-c
        # x/wqk chunks first
        x0 = xp.tile([P, NCT, TQ], BF16, tag="xc")
        src0 = xt[:, 0:TQ].rearrange("(c p) t -> p c t", p=P)
        nc.sync.dma_start(x0[:, 0:4, :], src0[:, 0:4, :])
        nc.sync.dma_start(wqk_sb[:, 0:2, :], wqk[:, 0:2, :])
        nc.sync.dma_start(wqk_sb[:, 2:4, :], wqk[:, 2:4, :])
        nc.sync.dma_start(x0[:, 4:8, :], src0[:, 4:8, :])
        nc.sync.dma_start(wqk_sb[:, 4:6, :], wqk[:, 4:6, :])
        nc.sync.dma_start(wqk_sb[:, 6:8, :], wqk[:, 6:8, :])
        nc.sync.dma_start(wv_sb, wv[:])
        nc.sync.dma_start(bqk_sb, bqk[:])
        nc.sync.dma_start(tri_sb, tri[:])
        nc.sync.dma_start(bv_sb[0:1, :], bv[:])
        nc.gpsimd.partition_broadcast(bv_sb[:, :], bv_sb[0:1, :])
        bv_hd = bv_sb.rearrange("p (h d) -> p h d", h=NHL)
        # ones columns of the augmented V
        nc.vector.memset(v_sb[:, :, :, HD : HD + 1], 1.0)

        s1_compute(0, x0)
        nc.sync.dma_start(wp_sb, wp[:])
        xs = {}
        for t in range(NIT):
            if t + 1 < NIT:
                xs[t + 1] = s1_load(t + 1)
            att_phase(t)
            if t + 1 < NIT:
                s1_compute(t + 1, xs[t + 1])
        # stage 3 last (lowest scheduler priority): it back-fills the PE
        # during the ACT-bound later attention phases, where the per-key-tile
        # cadence is set by the Scalar-engine exp and the PE has idle slots
        for tt in range(NTS):
            s3_tile(tt)

    nc.compile()
    return nc


_NC_CACHE = None


def _get_nc():
    global _NC_CACHE
    if _NC_CACHE is None:
        _NC_CACHE = build_kernel()
    return _NC_CACHE


def _shard_inputs(x, w_qkv, b_qkv, w_proj):
    """Build the 8 per-core input maps. Core id = 2*batch + head_group."""
    bf = ml_dtypes.bfloat16
    tri01 = np.where(
        np.arange(P)[None, :] >= np.arange(P)[:, None], 1.0, 0.0
    )
    tri_np = np.ascontiguousarray(
        np.stack([tri01, tri01], axis=1)
    ).astype(bf)  # [P, 2, P]

    # q/k m-block row permutation: m in 0..7 -> (q|k, hg=(m%4)//2, pl=m%2);
    # partition p = 32*gg + i  ->  local head 4*hg + gg, dim 32*pl + i
    perm = np.empty((8, P), np.int64)
    for m in range(8):
        base = 0 if m < 4 else 1024
        mm = m % 4
        hg, pl = mm // 2, mm % 2
        for gg in range(4):
            h = 4 * hg + gg
            perm[m, 32 * gg : 32 * gg + 32] = (
                base + 64 * h + 32 * pl + np.arange(32)
            )

    in_maps = []
    for b in range(B):
        xt = np.ascontiguousarray(x[b].T).astype(bf)  # [C, T]
        for g in range(2):
            s = slice(g * 512, (g + 1) * 512)
            goff = g * 512  # global row offset of this head group's q rows
            rows = perm + goff  # k rows offset by 1024 already in perm base
            wqk_full = w_qkv[rows.reshape(-1)]  # [1024 f, 1024 c]
            wqk_arr = np.ascontiguousarray(
                wqk_full.T.reshape(NCT, P, 1024).transpose(1, 0, 2)
            ).astype(bf)
            bqk_full = b_qkv[rows.reshape(-1)]
            bqk_arr = np.ascontiguousarray(bqk_full.reshape(8, P).T)
            wv_rows = w_qkv[2048:3072][s]  # [512 f, 1024 c]
            wv_arr = np.ascontiguousarray(
                wv_rows.T.reshape(NCT, P, 512).transpose(1, 0, 2)
            ).astype(bf)
            bv_arr = np.ascontiguousarray(b_qkv[2048:3072][s][None, :])
            wp_rhs = w_proj[:, s].T  # [512 hd, 1024 o]
            wp_arr = np.ascontiguousarray(
                wp_rhs.reshape(NPAIR, P, 1024).transpose(1, 0, 2)
            ).astype(bf)
            in_maps.append(
                {
                    "xt": xt,
                    "wqk": wqk_arr,
                    "bqk": bqk_arr.astype(np.float32),
                    "wv": wv_arr,
                    "bv": bv_arr.astype(np.float32),
                    "wp": wp_arr,
                    "tri": tri_np,
                }
            )
    return in_maps


def kernel(x, w_qkv, b_qkv, w_proj, b_proj, _trace=False, _trace_kwargs=None):
    x = np.asarray(x, dtype=np.float32)
    w_qkv = np.asarray(w_qkv, dtype=np.float32)
    b_qkv = np.asarray(b_qkv, dtype=np.float32)
    w_proj = np.asarray(w_proj, dtype=np.float32)
    b_proj = np.asarray(b_proj, dtype=np.float32)

    nc = _get_nc()
    in_maps = _shard_inputs(x, w_qkv, b_qkv, w_proj)
    res = run_bass_kernel_spmd(
        nc, in_maps, core_ids=list(range(8)), trace=_trace,
        **(_trace_kwargs or {}),
    )
    out = np.empty((B, T, C), np.float32)
    for b in range(B):
        out[b] = res.results[2 * b]["out"] + res.results[2 * b + 1]["out"] + b_proj
    if _trace:
        return out, res
    return out


# revision 25
# speedup vs baseline: 1.2865x; 1.2021x over previous
"""Causal self-attention (B=4, T=2048, C=1024, H=16) on 8 TRN2 NeuronCores.

Sharding: 8 cores = 4 batches x 2 head-groups (Megatron tensor-parallel over
heads + data-parallel over batch). Each core computes, for its batch b and its
8 heads, a fused flash-style pipeline over 512-token tiles t:

  phase t: attention for query tile t (all 4 head pairs, causal key tiles)
           interleaved by the Tile scheduler with stage-1 qkv projection of
           tile t+1; stage-3 output projection is emitted last (lowest
           priority) so it back-fills the PE during the ACT-bound later
           attention phases.

All stage-1/QK matmuls run as fp8e4m3 DoubleRow (2 contraction planes per
matmul, 0.5 cycles/column = 4x bf16 MACs):
  x is shipped from the host as a hi/lo fp8 pair (xh ~ x/4, xl ~ 16*(x-xh)),
  so the v projection runs as three DoubleRow terms xh*wvh + xh*wvl + xl*wvh4
  at ~bf16 precision, while the q/k projection uses the single xh*wqk8 term
  (its noise only perturbs logits, which the fp8 QK path tolerates anyway).
  All products land at 64x natural scale; the q/k store divides by 64 on the
  Vector engine, while v keeps the 64x scale with a 64-valued ones-column so
  the softmax normalization cancels it for free.
  q8/k8 [fp8; head h on partitions 32g..32g+31 (g=h%4), dim planes (0-31,
  32-63) adjacent on the free dim] feed fp8 DoubleRow QK^T logits;
  exp(x*0.125) on the Scalar engine applies the 1/sqrt(hd) scale for free.
  V [tokens on partitions, head-major dims on free, bf16].
  exp straight out of PSUM (no max-subtraction: logits are O(1) by
  construction); the causal triangle of diagonal blocks is zeroed by a 0/1
  bf16 mask multiply on the Vector engine after the exp. A short burst of
  dependency-free warm-up matmuls at t=0 ramps the PE p-state while the
  first x/w DMAs land.

Host: shards + fp8-splits inputs, sums the two partial outputs per batch,
adds b_proj.
"""

import sys

if "/opt/trn_rl_repo" not in sys.path:
    sys.path.insert(0, "/opt/trn_rl_repo")

from contextlib import ExitStack

import numpy as np
import ml_dtypes

import concourse.bass as bass
import concourse.tile as tile
from concourse import bacc, mybir
from concourse.bass_utils import run_bass_kernel_spmd

F32 = mybir.dt.float32
BF16 = mybir.dt.bfloat16
FP8 = mybir.dt.float8e4
AF = mybir.ActivationFunctionType
ADD = mybir.AluOpType.add
MULT = mybir.AluOpType.mult
DR = mybir.MatmulPerfMode.DoubleRow

B, T, C = 4, 2048, 1024
H, HD = 16, 64
NHL = 8          # heads per core (local)
NPAIR = 4        # head pairs per core
P = 128
TQ = 512         # query tile (free dim)
TJ = 128         # key tile (partitions)
NIT = T // TQ    # 4 query tiles
NTS = T // P     # 16 token sub-tiles
NCT = C // P     # 8 contraction tiles over C
NC2 = NCT // 2   # 4 DoubleRow contraction pairs
N_WARM = 28      # dep-free PE warm-up matmuls (64 cols each)
VSC = 64.0       # v/psum scale (cancelled by the 64-valued ones column)
M_ORDER = [0, 1, 4, 5, 2, 3, 6, 7]  # wqk column-block order (pairs 0-1 first)


def build_kernel():
    nc = bacc.Bacc("TRN2", target_bir_lowering=False)

    xh = nc.declare_dram_parameter("xh", [C, T], FP8, isOutput=False)
    xl = nc.declare_dram_parameter("xl", [C, T], FP8, isOutput=False)
    wqk8 = nc.declare_dram_parameter("wqk8", [P, NCT, 1024], FP8, isOutput=False)
    bqk = nc.declare_dram_parameter("bqk", [P, 8], F32, isOutput=False)
    wvh = nc.declare_dram_parameter("wvh", [P, NCT, 512], FP8, isOutput=False)
    wvl = nc.declare_dram_parameter("wvl", [P, NCT, 512], FP8, isOutput=False)
    wvh4 = nc.declare_dram_parameter("wvh4", [P, NCT, 512], FP8, isOutput=False)
    bv = nc.declare_dram_parameter("bv", [1, 512], F32, isOutput=False)
    wp = nc.declare_dram_parameter("wp", [P, NPAIR, 1024], BF16, isOutput=False)
    tri = nc.declare_dram_parameter("tri", [P, 2, P], BF16, isOutput=False)
    out = nc.declare_dram_parameter("out", [T, C], F32, isOutput=True)

    with tile.TileContext(nc) as tc, ExitStack() as ctx:
        persist = ctx.enter_context(tc.tile_pool(name="persist", bufs=1))
        xp = ctx.enter_context(tc.tile_pool(name="xp", bufs=4))
        attp = ctx.enter_context(tc.tile_pool(name="attp", bufs=20))
        rtp = ctx.enter_context(tc.tile_pool(name="rtp", bufs=6))
        rbp = ctx.enter_context(tc.tile_pool(name="rbp", bufs=6))
        otp = ctx.enter_context(tc.tile_pool(name="otp", bufs=3))
        accps = ctx.enter_context(tc.tile_pool(name="accps", bufs=2, space="PSUM"))
        qkps = ctx.enter_context(tc.tile_pool(name="qkps", bufs=2, space="PSUM"))
        pvps = ctx.enter_context(tc.tile_pool(name="pvps", bufs=2, space="PSUM"))

        # q8/k8: [part(32g+dim%32), hg, dim-plane, tok]; head h = 4*hg + g
        q8_sb = persist.tile([P, 2, 2, T], FP8)
        k8_sb = persist.tile([P, 2, 2, T], FP8)
        v_sb = persist.tile([P, NTS, NHL, HD + 1], BF16)
        y_sbs = [
            persist.tile([P, T], BF16, name=f"y{a}_sb") for a in range(NPAIR)
        ]
        wqk_sb = persist.tile([P, NCT, 1024], FP8)
        wvh_sb = persist.tile([P, NCT, 512], FP8)
        wvl_sb = persist.tile([P, NCT, 512], FP8)
        wvh4_sb = persist.tile([P, NCT, 512], FP8)
        wp_sb = persist.tile([P, NPAIR, 1024], BF16)
        bqk_sb = persist.tile([P, 8], F32)
        bv_sb = persist.tile([P, 512], F32)
        tri_sb = persist.tile([P, 2, P], BF16)
        warm_sb = persist.tile([P, 64], BF16)

        def s1_load(t, split=False):
            # one DMA per 512-token x tile half: [C_chunk on parts, chunk, tok]
            t0 = t * TQ
            xih = xp.tile([P, NCT, TQ], FP8, tag="xch")
            xil = xp.tile([P, NCT, TQ], FP8, tag="xcl")
            srch = xh[:, t0 : t0 + TQ].rearrange("(c p) t -> p c t", p=P)
            srcl = xl[:, t0 : t0 + TQ].rearrange("(c p) t -> p c t", p=P)
            if split:
                nc.sync.dma_start(xih[:, 0:4, :], srch[:, 0:4, :])
                nc.sync.dma_start(xih[:, 4:8, :], srch[:, 4:8, :])
                nc.sync.dma_start(xil[:, 0:4, :], srcl[:, 0:4, :])
                nc.sync.dma_start(xil[:, 4:8, :], srcl[:, 4:8, :])
            else:
                nc.sync.dma_start(xih, srch)
                nc.sync.dma_start(xil, srcl)
            return xih, xil

        def s1_qk_block(t, xih, mi):
            # m-block M_ORDER[mi] lives at column block mi of the reordered
            # wqk; blocks 0-3 cover heads 0-3 (hg=0) for the first two pairs
            t0 = t * TQ
            m = M_ORDER[mi]
            ps = accps.tile([P, TQ], F32, tag="acc")
            for c2 in range(NC2):
                nc.tensor.matmul(
                    ps,
                    wqk_sb[:, 2 * c2 : 2 * c2 + 2, mi * P : (mi + 1) * P],
                    xih[:, 2 * c2 : 2 * c2 + 2, :],
                    start=(c2 == 0),
                    stop=(c2 == NC2 - 1),
                    perf_mode=DR,
                )
            dst = q8_sb if m < 4 else k8_sb
            mm = m % 4
            nc.vector.tensor_scalar(
                dst[:, mm // 2, mm % 2, t0 : t0 + TQ], ps,
                bqk_sb[:, m : m + 1], 1.0 / VSC, ADD, MULT,
            )

        def s1_v_block(t, xih, xil, s):
            # one v block: out [t-part, f-free(head-major)]; three fp8 terms
            # (hi*hi + hi*lo + lo*hi) for ~bf16 accuracy on the value path
            ps = accps.tile([P, 512], F32, tag="acc")
            for c2 in range(NC2):
                sx = slice(2 * c2, 2 * c2 + 2)
                st = slice(s * P, (s + 1) * P)
                nc.tensor.matmul(
                    ps, xih[:, sx, st], wvh_sb[:, sx, :],
                    start=(c2 == 0), stop=False, perf_mode=DR,
                )
                nc.tensor.matmul(
                    ps, xih[:, sx, st], wvl_sb[:, sx, :],
                    start=False, stop=False, perf_mode=DR,
                )
                nc.tensor.matmul(
                    ps, xil[:, sx, st], wvh4_sb[:, sx, :],
                    start=False, stop=(c2 == NC2 - 1), perf_mode=DR,
                )
            tsub = t * (TQ // P) + s
            nc.vector.tensor_tensor(
                v_sb[:, tsub, :, 0:HD],
                ps.rearrange("p (h d) -> p h d", h=NHL),
                bv_hd,
                ADD,
            )

        def s1_compute(t, xih, xil):
            # q/k blocks for pairs 0-1 first (they gate the first exps),
            # then v (gates PV), then the pair-2/3 q/k blocks
            for mi in range(4):
                s1_qk_block(t, xih, mi)
            for s in range(TQ // P):
                s1_v_block(t, xih, xil, s)
            for mi in range(4, 8):
                s1_qk_block(t, xih, mi)

        def qk_exp(it, a, jt):
            # one (pair, key-tile) unit: 2 DoubleRow QK matmuls + exp
            i0 = it * TQ
            hg = a // 2
            j0 = jt * TJ
            d = j0 - i0
            istart = max(d, 0)
            nn = TQ - istart
            qk = qkps.tile([P, 2, TQ], F32, tag="qk")
            for e in (0, 1):
                bp = 32 * (2 * (a % 2) + e)
                nc.tensor.matmul(
                    qk[:, e, istart:TQ],
                    k8_sb[bp : bp + 32, hg, :, j0 : j0 + TJ],
                    q8_sb[bp : bp + 32, hg, :, i0 + istart : i0 + TQ],
                    start=True,
                    stop=True,
                    perf_mode=DR,
                    tile_position=(bp, 0),
                )
            att = attp.tile([P, 2, TQ], BF16, tag="att")
            nc.scalar.activation(
                att[:, :, 0:nn], qk[:, :, istart:TQ], AF.Exp, scale=0.125
            )
            if d >= 0:
                # zero the upper triangle of the diagonal block off the
                # PE's critical path
                nc.vector.tensor_tensor(
                    att[:, :, 0:TJ], att[:, :, 0:TJ], tri_sb,
                    mybir.AluOpType.mult,
                )
            return att, istart, nn

        def att_phase(it, fills=None, init_units=None, hoist_next=False):
            i0 = it * TQ
            njt = (i0 + TQ) // TJ
            fills = list(fills or [])
            points = [j for j in (2, 5, 8, 11, 14) if j < njt]
            pts_left = len(points) * NPAIR
            # phase 0: emit every QK+exp unit before any PV so the exp
            # stream never sits behind PV's wait on the (late) v DMAs
            allunits = None
            if it == 0:
                allunits = {
                    (a, jt): qk_exp(it, a, jt)
                    for a in range(NPAIR)
                    for jt in range(njt)
                }
            pend = {0: list(init_units or [])}
            ret = None
            for a in range(NPAIR):
                pv0 = pvps.tile([P, TQ], F32, tag="pv", name=f"pv0_{a}_{it}")
                pv1 = pvps.tile([P, TQ], F32, tag="pv", name=f"pv1_{a}_{it}")
                units = pend.pop(a, [])
                for jt in range(njt):
                    if allunits is not None:
                        att, istart, nn = allunits[(a, jt)]
                    else:
                        # keep the QK emission two key-tiles ahead of the PV
                        # consumption (matching the double-buffered QK PSUM)
                        # so fresh logits are always interleaved ahead of the
                        # PV backlog in the engine's static order
                        while len(units) <= min(jt + 4, njt - 1):
                            units.append(qk_exp(it, a, len(units)))
                        att, istart, nn = units[jt]
                    last = jt == njt - 1
                    nc.tensor.matmul(
                        pv0[0 : HD + 1, istart:TQ],
                        v_sb[:, jt, 2 * a, :],
                        att[:, 0, 0:nn],
                        start=(jt == 0),
                        stop=last,
                    )
                    nc.tensor.matmul(
                        pv1[0 : HD + 1, istart:TQ],
                        v_sb[:, jt, 2 * a + 1, :],
                        att[:, 1, 0:nn],
                        start=(jt == 0),
                        stop=last,
                    )
                    # spread PE back-fill (next tile's projections, older
                    # tokens' output projection) evenly across the phase,
                    # never at a pair boundary
                    if jt in points:
                        if fills:
                            k = -(-len(fills) // pts_left)
                            for _ in range(min(k, len(fills))):
                                fills.pop(0)()
                        pts_left -= 1
                # hoist the next pair's first two QK+exp units ahead of the
                # normalize chain so the Scalar engine never idles across
                # the pair boundary
                if allunits is None and a + 1 < NPAIR:
                    pend[a + 1] = [qk_exp(it, a + 1, 0), qk_exp(it, a + 1, 1)]
                if a == NPAIR - 1 and hoist_next:
                    # same trick across the phase boundary (the next tile's
                    # q/k blocks were back-filled early in this phase)
                    ret = [qk_exp(it + 1, 0, 0), qk_exp(it + 1, 0, 1)]
                # normalize: softmax sums sit at row HD of each pv tile (the
                # 64x v scale cancels against the 64-valued ones column); the
                # odd head (whose y needs a partition-move DMA) goes first so
                # its longer chain overlaps the even head's
                rt1 = rtp.tile([P, TQ], F32, tag="rt")
                rb1 = rbp.tile([P, TQ], F32, tag="rb")
                nc.vector.reciprocal(rt1[HD : HD + 1, :], pv1[HD : HD + 1, :])
                nc.sync.dma_start(rb1[0:1, :], rt1[HD : HD + 1, :])
                nc.gpsimd.partition_broadcast(rb1[0:HD, :], rb1[0:1, :])
                rt = rtp.tile([P, TQ], F32, tag="rt")
                rb = rbp.tile([P, TQ], F32, tag="rb")
                nc.vector.reciprocal(rt[HD : HD + 1, :], pv0[HD : HD + 1, :])
                nc.sync.dma_start(rb[0:1, :], rt[HD : HD + 1, :])
                nc.gpsimd.partition_broadcast(rb[0:HD, :], rb[0:1, :])
                if it == NIT - 1 and a == NPAIR - 1:
                    # final pair: normalize per 128-token chunk and run each
                    # remaining output-projection tile as soon as its chunk
                    # lands, collapsing the kernel tail
                    for c in range(TQ // P):
                        sl = slice(i0 + c * P, i0 + (c + 1) * P)
                        cc = slice(c * P, (c + 1) * P)
                        ytc = rtp.tile([P, P], BF16, tag="ytc")
                        nc.vector.tensor_mul(
                            ytc[0:HD, :], pv1[0:HD, cc], rb1[0:HD, cc]
                        )
                        nc.sync.dma_start(y_sbs[a][HD:P, sl], ytc[0:HD, :])
                        nc.vector.tensor_mul(
                            y_sbs[a][0:HD, sl], pv0[0:HD, cc], rb[0:HD, cc]
                        )
                        s3_tile(4 * it + c)
                else:
                    yt = rtp.tile([P, TQ], BF16, tag="yt")
                    nc.vector.tensor_mul(yt[0:HD, :], pv1[0:HD, :], rb1[0:HD, :])
                    nc.sync.dma_start(y_sbs[a][HD:P, i0 : i0 + TQ], yt[0:HD, :])
                    nc.vector.tensor_mul(
                        y_sbs[a][0:HD, i0 : i0 + TQ], pv0[0:HD, :], rb[0:HD, :]
                    )
            for f in fills:
                f()
            return ret

        def s3_tile(tt, fine=True):
            ot_sb = otp.tile([P, 1024], F32, tag="osb")
            for ot in range(2):
                ps = accps.tile([P, 512], F32, tag="acc")
                for a in range(NPAIR):
                    nc.tensor.matmul(
                        ps,
                        y_sbs[a][:, tt * P : (tt + 1) * P],
                        wp_sb[:, a, ot * 512 : (ot + 1) * 512],
                        start=(a == 0),
                        stop=(a == NPAIR - 1),
                    )
                nc.vector.tensor_copy(ot_sb[:, ot * 512 : (ot + 1) * 512], ps)
                # ship each half as soon as its copy lands
                nc.sync.dma_start(
                    out[tt * P : (tt + 1) * P, ot * 512 : (ot + 1) * 512],
                    ot_sb[:, ot * 512 : (ot + 1) * 512],
                )

        # ---------------- fused pipeline ----------------
        # PE warm-up: dependency-free matmuls start the p-state ramp at t~0
        # and keep the PE busy while the first x/wqk DMAs land.
        nc.vector.memset(warm_sb, 1.0)
        for _ in range(N_WARM):
            wps = accps.tile([P, TQ], F32, tag="acc")
            nc.tensor.matmul(
                wps[0:64, 0:64], warm_sb[:, 0:64], warm_sb[:, 0:64],
                start=True, stop=True,
            )
        # head: x(0) + weights interleaved finely — the DMA device transfers
        # in issue order, and the first m-chunk accumulations need the low-c
        # x/wqk chunks first
        x0h = xp.tile([P, NCT, TQ], FP8, tag="xch")
        x0l = xp.tile([P, NCT, TQ], FP8, tag="xcl")
        src0h = xh[:, 0:TQ].rearrange("(c p) t -> p c t", p=P)
        src0l = xl[:, 0:TQ].rearrange("(c p) t -> p c t", p=P)
        nc.sync.dma_start(x0h[:, 0:4, :], src0h[:, 0:4, :])
        nc.sync.dma_start(x0h[:, 4:8, :], src0h[:, 4:8, :])
        nc.sync.dma_start(bqk_sb, bqk[:])
        nc.sync.dma_start(wqk_sb[:, :, 0:512], wqk8[:, :, 0:512])
        nc.sync.dma_start(tri_sb, tri[:])
        nc.sync.dma_start(x0l, src0l)
        nc.sync.dma_start(wvh_sb, wvh[:])
        nc.sync.dma_start(wqk_sb[:, :, 512:1024], wqk8[:, :, 512:1024])
        nc.sync.dma_start(wvl_sb, wvl[:])
        nc.sync.dma_start(wvh4_sb, wvh4[:])
        nc.sync.dma_start(bv_sb[0:1, :], bv[:])
        nc.gpsimd.partition_broadcast(bv_sb[:, :], bv_sb[0:1, :])
        bv_hd = bv_sb.rearrange("p (h d) -> p h d", h=NHL)
        # ones columns of the augmented V carry the 64x v scale
        nc.vector.memset(v_sb[:, :, :, HD : HD + 1], VSC)

        s1_compute(0, x0h, x0l)
        nc.sync.dma_start(wp_sb, wp[:])
        # stage-3 back-fill: older tokens' projections run inside the later
        # (ACT-bound) phases; the last phase's own tiles go at the end
        S3_FILL = {2: (0, 1, 2, 3), 3: (4, 5, 6, 7, 8, 9, 10, 11)}
        units0 = None
        for t in range(NIT):
            fills = []
            if t + 1 < NIT:
                xih, xil = s1_load(t + 1)
                for mi in range(4):
                    fills.append(
                        lambda t1=t + 1, x_=xih, mi_=mi: s1_qk_block(t1, x_, mi_)
                    )
                for s in range(TQ // P):
                    fills.append(
                        lambda t1=t + 1, a_=xih, b_=xil, s_=s: s1_v_block(
                            t1, a_, b_, s_
                        )
                    )
                for mi in range(4, 8):
                    fills.append(
                        lambda t1=t + 1, x_=xih, mi_=mi: s1_qk_block(t1, x_, mi_)
                    )
            for tt in S3_FILL.get(t, ()):
                fills.append(lambda tt_=tt: s3_tile(tt_))
            units0 = att_phase(
                t, fills, init_units=units0, hoist_next=(t + 1 < NIT)
            )
    nc.compile()
    return nc


_NC_CACHE = None


def _get_nc():
    global _NC_CACHE
    if _NC_CACHE is None:
        _NC_CACHE = build_kernel()
    return _NC_CACHE


def _fp8(a):
    return np.asarray(a, np.float32).astype(ml_dtypes.float8_e4m3)


def _shard_inputs(x, w_qkv, b_qkv, w_proj):
    """Build the 8 per-core input maps. Core id = 2*batch + head_group."""
    bf = ml_dtypes.bfloat16
    tri01 = np.where(
        np.arange(P)[None, :] >= np.arange(P)[:, None], 1.0, 0.0
    )
    tri_np = np.ascontiguousarray(
        np.stack([tri01, tri01], axis=1)
    ).astype(bf)  # [P, 2, P]

    # q/k m-block row permutation: m in 0..7 -> (q|k, hg=(m%4)//2, pl=m%2);
    # partition p = 32*gg + i  ->  local head 4*hg + gg, dim 32*pl + i
    perm = np.empty((8, P), np.int64)
    for m in range(8):
        base = 0 if m < 4 else 1024
        mm = m % 4
        hg, pl = mm // 2, mm % 2
        for gg in range(4):
            h = 4 * hg + gg
            perm[m, 32 * gg : 32 * gg + 32] = (
                base + 64 * h + 32 * pl + np.arange(32)
            )

    def pack_w(rows_w, width):
        # [width f, 1024 c] -> [P part(c%128), NCT, width]
        return np.ascontiguousarray(
            rows_w.T.reshape(NCT, P, width).transpose(1, 0, 2)
        )

    in_maps = []
    for b in range(B):
        xt = np.ascontiguousarray(x[b].T)          # [C, T] f32
        xh_q = _fp8(xt * 0.25)                     # hi: x/4
        xl_q = _fp8((xt - xh_q.astype(np.float32) * 4.0) * 16.0)  # lo: 16*dx
        for g in range(2):
            s = slice(g * 512, (g + 1) * 512)
            rows = perm + g * 512
            wqk_full = w_qkv[rows[M_ORDER].reshape(-1)]  # [1024 f, 1024 c]
            wqk8_arr = _fp8(pack_w(wqk_full, 1024) * 256.0)
            bqk_full = b_qkv[rows.reshape(-1)] * VSC  # indexed by m, not mi
            bqk_arr = np.ascontiguousarray(bqk_full.reshape(8, P).T)
            wv_rows = w_qkv[2048:3072][s]          # [512 f, 1024 c]
            wvh_arr = _fp8(pack_w(wv_rows, 512) * 256.0)
            dv = pack_w(wv_rows, 512) - wvh_arr.astype(np.float32) / 256.0
            wvl_arr = _fp8(dv * 256.0)
            wvh4_arr = _fp8(pack_w(wv_rows, 512) * 4.0)
            bv_arr = np.ascontiguousarray(
                b_qkv[2048:3072][s][None, :] * VSC
            )
            wp_rhs = w_proj[:, s].T                # [512 hd, 1024 o]
            wp_arr = np.ascontiguousarray(
                wp_rhs.reshape(NPAIR, P, 1024).transpose(1, 0, 2)
            ).astype(bf)
            in_maps.append(
                {
                    "xh": xh_q,
                    "xl": xl_q,
                    "wqk8": wqk8_arr,
                    "bqk": bqk_arr.astype(np.float32),
                    "wvh": wvh_arr,
                    "wvl": wvl_arr,
                    "wvh4": wvh4_arr,
                    "bv": bv_arr.astype(np.float32),
                    "wp": wp_arr,
                    "tri": tri_np,
                }
            )
    return in_maps


def kernel(x, w_qkv, b_qkv, w_proj, b_proj, _trace=False, _trace_kwargs=None):
    x = np.asarray(x, dtype=np.float32)
    w_qkv = np.asarray(w_qkv, dtype=np.float32)
    b_qkv = np.asarray(b_qkv, dtype=np.float32)
    w_proj = np.asarray(w_proj, dtype=np.float32)
    b_proj = np.asarray(b_proj, dtype=np.float32)

    nc = _get_nc()
    in_maps = _shard_inputs(x, w_qkv, b_qkv, w_proj)
    res = run_bass_kernel_spmd(
        nc, in_maps, core_ids=list(range(8)), trace=_trace,
        **(_trace_kwargs or {}),
    )
    out = np.empty((B, T, C), np.float32)
    for b in range(B):
        out[b] = res.results[2 * b]["out"] + res.results[2 * b + 1]["out"] + b_proj
    if _trace:
        return out, res
    return out


# revision 26
# speedup vs baseline: 1.2868x; 1.0003x over previous
"""Causal self-attention (B=4, T=2048, C=1024, H=16) on 8 TRN2 NeuronCores.

Sharding: 8 cores = 4 batches x 2 head-groups (Megatron tensor-parallel over
heads + data-parallel over batch). Each core computes, for its batch b and its
8 heads, a fused flash-style pipeline over 512-token tiles t:

  phase t: attention for query tile t (all 4 head pairs, causal key tiles)
           interleaved by the Tile scheduler with stage-1 qkv projection of
           tile t+1; stage-3 output projection is emitted last (lowest
           priority) so it back-fills the PE during the ACT-bound later
           attention phases.

All stage-1/QK matmuls run as fp8e4m3 DoubleRow (2 contraction planes per
matmul, 0.5 cycles/column = 4x bf16 MACs):
  x is shipped from the host as a hi/lo fp8 pair (xh ~ x/4, xl ~ 16*(x-xh)),
  so the v projection runs as three DoubleRow terms xh*wvh + xh*wvl + xl*wvh4
  at ~bf16 precision, while the q/k projection uses the single xh*wqk8 term
  (its noise only perturbs logits, which the fp8 QK path tolerates anyway).
  All products land at 64x natural scale; the q/k store divides by 64 on the
  Vector engine, while v keeps the 64x scale with a 64-valued ones-column so
  the softmax normalization cancels it for free.
  q8/k8 [fp8; head h on partitions 32g..32g+31 (g=h%4), dim planes (0-31,
  32-63) adjacent on the free dim] feed fp8 DoubleRow QK^T logits;
  exp(x*0.125) on the Scalar engine applies the 1/sqrt(hd) scale for free.
  V [tokens on partitions, head-major dims on free, bf16].
  exp straight out of PSUM (no max-subtraction: logits are O(1) by
  construction); the causal triangle of diagonal blocks is zeroed by a 0/1
  bf16 mask multiply on the Vector engine after the exp. A short burst of
  dependency-free warm-up matmuls at t=0 ramps the PE p-state while the
  first x/w DMAs land.

Host: shards + fp8-splits inputs, sums the two partial outputs per batch,
adds b_proj.
"""

import sys

if "/opt/trn_rl_repo" not in sys.path:
    sys.path.insert(0, "/opt/trn_rl_repo")

from contextlib import ExitStack

import numpy as np
import ml_dtypes

import concourse.bass as bass
import concourse.tile as tile
from concourse import bacc, mybir
from concourse.bass_utils import run_bass_kernel_spmd

F32 = mybir.dt.float32
BF16 = mybir.dt.bfloat16
FP8 = mybir.dt.float8e4
AF = mybir.ActivationFunctionType
ADD = mybir.AluOpType.add
MULT = mybir.AluOpType.mult
DR = mybir.MatmulPerfMode.DoubleRow

B, T, C = 4, 2048, 1024
H, HD = 16, 64
NHL = 8          # heads per core (local)
NPAIR = 4        # head pairs per core
P = 128
TQ = 512         # query tile (free dim)
TJ = 128         # key tile (partitions)
NIT = T // TQ    # 4 query tiles
NTS = T // P     # 16 token sub-tiles
NCT = C // P     # 8 contraction tiles over C
NC2 = NCT // 2   # 4 DoubleRow contraction pairs
N_WARM = 28      # dep-free PE warm-up matmuls (64 cols each)
VSC = 64.0       # v/psum scale (cancelled by the 64-valued ones column)
M_ORDER = [0, 1, 4, 5, 2, 3, 6, 7]  # wqk column-block order (pairs 0-1 first)


def build_kernel():
    nc = bacc.Bacc("TRN2", target_bir_lowering=False)

    xh = nc.declare_dram_parameter("xh", [C, T], FP8, isOutput=False)
    xl = nc.declare_dram_parameter("xl", [C, T], FP8, isOutput=False)
    wqk8 = nc.declare_dram_parameter("wqk8", [P, NCT, 1024], FP8, isOutput=False)
    bqk = nc.declare_dram_parameter("bqk", [P, 8], F32, isOutput=False)
    wvh = nc.declare_dram_parameter("wvh", [P, NCT, 512], FP8, isOutput=False)
    wvl = nc.declare_dram_parameter("wvl", [P, NCT, 512], FP8, isOutput=False)
    wvh4 = nc.declare_dram_parameter("wvh4", [P, NCT, 512], FP8, isOutput=False)
    bv = nc.declare_dram_parameter("bv", [1, 512], F32, isOutput=False)
    wp = nc.declare_dram_parameter("wp", [P, NPAIR, 1024], BF16, isOutput=False)
    tri = nc.declare_dram_parameter("tri", [P, 2, P], BF16, isOutput=False)
    out = nc.declare_dram_parameter("out", [T, C], F32, isOutput=True)

    with tile.TileContext(nc) as tc, ExitStack() as ctx:
        persist = ctx.enter_context(tc.tile_pool(name="persist", bufs=1))
        xp = ctx.enter_context(tc.tile_pool(name="xp", bufs=4))
        attp = ctx.enter_context(tc.tile_pool(name="attp", bufs=20))
        rtp = ctx.enter_context(tc.tile_pool(name="rtp", bufs=6))
        rbp = ctx.enter_context(tc.tile_pool(name="rbp", bufs=6))
        otp = ctx.enter_context(tc.tile_pool(name="otp", bufs=3))
        accps = ctx.enter_context(tc.tile_pool(name="accps", bufs=2, space="PSUM"))
        qkps = ctx.enter_context(tc.tile_pool(name="qkps", bufs=2, space="PSUM"))
        pvps = ctx.enter_context(tc.tile_pool(name="pvps", bufs=2, space="PSUM"))

        # q8/k8: [part(32g+dim%32), hg, dim-plane, tok]; head h = 4*hg + g
        q8_sb = persist.tile([P, 2, 2, T], FP8)
        k8_sb = persist.tile([P, 2, 2, T], FP8)
        v_sb = persist.tile([P, NTS, NHL, HD + 1], BF16)
        y_sbs = [
            persist.tile([P, T], BF16, name=f"y{a}_sb") for a in range(NPAIR)
        ]
        wqk_sb = persist.tile([P, NCT, 1024], FP8)
        wvh_sb = persist.tile([P, NCT, 512], FP8)
        wvl_sb = persist.tile([P, NCT, 512], FP8)
        wvh4_sb = persist.tile([P, NCT, 512], FP8)
        wp_sb = persist.tile([P, NPAIR, 1024], BF16)
        bqk_sb = persist.tile([P, 8], F32)
        bv_sb = persist.tile([P, 512], F32)
        tri_sb = persist.tile([P, 2, P], BF16)
        warm_sb = persist.tile([P, 64], BF16)

        def s1_load(t, split=False):
            # one DMA per 512-token x tile half: [C_chunk on parts, chunk, tok]
            t0 = t * TQ
            xih = xp.tile([P, NCT, TQ], FP8, tag="xch")
            xil = xp.tile([P, NCT, TQ], FP8, tag="xcl")
            srch = xh[:, t0 : t0 + TQ].rearrange("(c p) t -> p c t", p=P)
            srcl = xl[:, t0 : t0 + TQ].rearrange("(c p) t -> p c t", p=P)
            if split:
                nc.sync.dma_start(xih[:, 0:4, :], srch[:, 0:4, :])
                nc.sync.dma_start(xih[:, 4:8, :], srch[:, 4:8, :])
                nc.sync.dma_start(xil[:, 0:4, :], srcl[:, 0:4, :])
                nc.sync.dma_start(xil[:, 4:8, :], srcl[:, 4:8, :])
            else:
                nc.sync.dma_start(xih, srch)
                nc.sync.dma_start(xil, srcl)
            return xih, xil

        def s1_qk_block(t, xih, mi):
            # m-block M_ORDER[mi] lives at column block mi of the reordered
            # wqk; blocks 0-3 cover heads 0-3 (hg=0) for the first two pairs
            t0 = t * TQ
            m = M_ORDER[mi]
            ps = accps.tile([P, TQ], F32, tag="acc")
            for c2 in range(NC2):
                nc.tensor.matmul(
                    ps,
                    wqk_sb[:, 2 * c2 : 2 * c2 + 2, mi * P : (mi + 1) * P],
                    xih[:, 2 * c2 : 2 * c2 + 2, :],
                    start=(c2 == 0),
                    stop=(c2 == NC2 - 1),
                    perf_mode=DR,
                )
            dst = q8_sb if m < 4 else k8_sb
            mm = m % 4
            nc.vector.tensor_scalar(
                dst[:, mm // 2, mm % 2, t0 : t0 + TQ], ps,
                bqk_sb[:, m : m + 1], 1.0 / VSC, ADD, MULT,
            )

        def s1_v_block(t, xih, xil, s):
            # one v block: out [t-part, f-free(head-major)]; three fp8 terms
            # (hi*hi + hi*lo + lo*hi) for ~bf16 accuracy on the value path
            ps = accps.tile([P, 512], F32, tag="acc")
            for c2 in range(NC2):
                sx = slice(2 * c2, 2 * c2 + 2)
                st = slice(s * P, (s + 1) * P)
                nc.tensor.matmul(
                    ps, xih[:, sx, st], wvh_sb[:, sx, :],
                    start=(c2 == 0), stop=False, perf_mode=DR,
                )
                nc.tensor.matmul(
                    ps, xih[:, sx, st], wvl_sb[:, sx, :],
                    start=False, stop=False, perf_mode=DR,
                )
                nc.tensor.matmul(
                    ps, xil[:, sx, st], wvh4_sb[:, sx, :],
                    start=False, stop=(c2 == NC2 - 1), perf_mode=DR,
                )
            tsub = t * (TQ // P) + s
            nc.vector.tensor_tensor(
                v_sb[:, tsub, :, 0:HD],
                ps.rearrange("p (h d) -> p h d", h=NHL),
                bv_hd,
                ADD,
            )

        def s1_compute(t, xih, xil):
            # q/k blocks for pairs 0-1 first (they gate the first exps),
            # then v (gates PV), then the pair-2/3 q/k blocks
            for mi in range(4):
                s1_qk_block(t, xih, mi)
            for s in range(TQ // P):
                s1_v_block(t, xih, xil, s)
            for mi in range(4, 8):
                s1_qk_block(t, xih, mi)

        def qk_exp(it, a, jt):
            # one (pair, key-tile) unit: 2 DoubleRow QK matmuls + exp
            i0 = it * TQ
            hg = a // 2
            j0 = jt * TJ
            d = j0 - i0
            istart = max(d, 0)
            nn = TQ - istart
            qk = qkps.tile([P, 2, TQ], F32, tag="qk")
            for e in (0, 1):
                bp = 32 * (2 * (a % 2) + e)
                nc.tensor.matmul(
                    qk[:, e, istart:TQ],
                    k8_sb[bp : bp + 32, hg, :, j0 : j0 + TJ],
                    q8_sb[bp : bp + 32, hg, :, i0 + istart : i0 + TQ],
                    start=True,
                    stop=True,
                    perf_mode=DR,
                    tile_position=(bp, 0),
                )
            att = attp.tile([P, 2, TQ], BF16, tag="att")
            nc.scalar.activation(
                att[:, :, 0:nn], qk[:, :, istart:TQ], AF.Exp, scale=0.125
            )
            if d >= 0:
                # zero the upper triangle of the diagonal block off the
                # PE's critical path
                nc.vector.tensor_tensor(
                    att[:, :, 0:TJ], att[:, :, 0:TJ], tri_sb,
                    mybir.AluOpType.mult,
                )
            return att, istart, nn

        def att_phase(it, fills=None, init_units=None, hoist_next=False):
            i0 = it * TQ
            njt = (i0 + TQ) // TJ
            fills = list(fills or [])
            points = [j for j in (2, 5, 8, 11, 14) if j < njt]
            pts_left = len(points) * NPAIR
            # phase 0: emit every QK+exp unit before any PV so the exp
            # stream never sits behind PV's wait on the (late) v DMAs
            allunits = None
            if it == 0:
                allunits = {
                    (a, jt): qk_exp(it, a, jt)
                    for a in range(NPAIR)
                    for jt in range(njt)
                }
            pend = {0: list(init_units or [])}
            ret = None
            for a in range(NPAIR):
                pv0 = pvps.tile([P, TQ], F32, tag="pv", name=f"pv0_{a}_{it}")
                pv1 = pvps.tile([P, TQ], F32, tag="pv", name=f"pv1_{a}_{it}")
                units = pend.pop(a, [])
                for jt in range(njt):
                    if allunits is not None:
                        att, istart, nn = allunits[(a, jt)]
                    else:
                        # keep the QK emission two key-tiles ahead of the PV
                        # consumption (matching the double-buffered QK PSUM)
                        # so fresh logits are always interleaved ahead of the
                        # PV backlog in the engine's static order
                        while len(units) <= min(jt + 8, njt - 1):
                            units.append(qk_exp(it, a, len(units)))
                        att, istart, nn = units[jt]
                    last = jt == njt - 1
                    nc.tensor.matmul(
                        pv0[0 : HD + 1, istart:TQ],
                        v_sb[:, jt, 2 * a, :],
                        att[:, 0, 0:nn],
                        start=(jt == 0),
                        stop=last,
                    )
                    nc.tensor.matmul(
                        pv1[0 : HD + 1, istart:TQ],
                        v_sb[:, jt, 2 * a + 1, :],
                        att[:, 1, 0:nn],
                        start=(jt == 0),
                        stop=last,
                    )
                    # spread PE back-fill (next tile's projections, older
                    # tokens' output projection) evenly across the phase,
                    # never at a pair boundary
                    if jt in points:
                        if fills:
                            k = -(-len(fills) // pts_left)
                            for _ in range(min(k, len(fills))):
                                fills.pop(0)()
                        pts_left -= 1
                # hoist the next pair's first two QK+exp units ahead of the
                # normalize chain so the Scalar engine never idles across
                # the pair boundary
                if allunits is None and a + 1 < NPAIR:
                    pend[a + 1] = [qk_exp(it, a + 1, 0), qk_exp(it, a + 1, 1)]
                if a == NPAIR - 1 and hoist_next:
                    # same trick across the phase boundary (the next tile's
                    # q/k blocks were back-filled early in this phase)
                    ret = [qk_exp(it + 1, 0, 0), qk_exp(it + 1, 0, 1)]
                # normalize: softmax sums sit at row HD of each pv tile (the
                # 64x v scale cancels against the 64-valued ones column); the
                # odd head (whose y needs a partition-move DMA) goes first so
                # its longer chain overlaps the even head's
                rt1 = rtp.tile([P, TQ], F32, tag="rt")
                rb1 = rbp.tile([P, TQ], F32, tag="rb")
                nc.vector.reciprocal(rt1[HD : HD + 1, :], pv1[HD : HD + 1, :])
                nc.sync.dma_start(rb1[0:1, :], rt1[HD : HD + 1, :])
                nc.gpsimd.partition_broadcast(rb1[0:HD, :], rb1[0:1, :])
                rt = rtp.tile([P, TQ], F32, tag="rt")
                rb = rbp.tile([P, TQ], F32, tag="rb")
                nc.vector.reciprocal(rt[HD : HD + 1, :], pv0[HD : HD + 1, :])
                nc.sync.dma_start(rb[0:1, :], rt[HD : HD + 1, :])
                nc.gpsimd.partition_broadcast(rb[0:HD, :], rb[0:1, :])
                if it == NIT - 1 and a == NPAIR - 1:
                    # final pair: normalize per 128-token chunk and run each
                    # remaining output-projection tile as soon as its chunk
                    # lands, collapsing the kernel tail
                    for c in range(TQ // P):
                        sl = slice(i0 + c * P, i0 + (c + 1) * P)
                        cc = slice(c * P, (c + 1) * P)
                        ytc = rtp.tile([P, P], BF16, tag="ytc")
                        nc.vector.tensor_mul(
                            ytc[0:HD, :], pv1[0:HD, cc], rb1[0:HD, cc]
                        )
                        nc.sync.dma_start(y_sbs[a][HD:P, sl], ytc[0:HD, :])
                        nc.vector.tensor_mul(
                            y_sbs[a][0:HD, sl], pv0[0:HD, cc], rb[0:HD, cc]
                        )
                        s3_tile(4 * it + c)
                else:
                    yt = rtp.tile([P, TQ], BF16, tag="yt")
                    nc.vector.tensor_mul(yt[0:HD, :], pv1[0:HD, :], rb1[0:HD, :])
                    nc.sync.dma_start(y_sbs[a][HD:P, i0 : i0 + TQ], yt[0:HD, :])
                    nc.vector.tensor_mul(
                        y_sbs[a][0:HD, i0 : i0 + TQ], pv0[0:HD, :], rb[0:HD, :]
                    )
            for f in fills:
                f()
            return ret

        def s3_tile(tt, fine=True):
            ot_sb = otp.tile([P, 1024], F32, tag="osb")
            for ot in range(2):
                ps = accps.tile([P, 512], F32, tag="acc")
                for a in range(NPAIR):
                    nc.tensor.matmul(
                        ps,
                        y_sbs[a][:, tt * P : (tt + 1) * P],
                        wp_sb[:, a, ot * 512 : (ot + 1) * 512],
                        start=(a == 0),
                        stop=(a == NPAIR - 1),
                    )
                nc.vector.tensor_copy(ot_sb[:, ot * 512 : (ot + 1) * 512], ps)
                # ship each half as soon as its copy lands
                nc.sync.dma_start(
                    out[tt * P : (tt + 1) * P, ot * 512 : (ot + 1) * 512],
                    ot_sb[:, ot * 512 : (ot + 1) * 512],
                )

        # ---------------- fused pipeline ----------------
        # PE warm-up: dependency-free matmuls start the p-state ramp at t~0
        # and keep the PE busy while the first x/wqk DMAs land.
        nc.vector.memset(warm_sb, 1.0)
        for _ in range(N_WARM):
            wps = accps.tile([P, TQ], F32, tag="acc")
            nc.tensor.matmul(
                wps[0:64, 0:64], warm_sb[:, 0:64], warm_sb[:, 0:64],
                start=True, stop=True,
            )
        # head: x(0) + weights interleaved finely — the DMA device transfers
        # in issue order, and the first m-chunk accumulations need the low-c
        # x/wqk chunks first
        x0h = xp.tile([P, NCT, TQ], FP8, tag="xch")
        x0l = xp.tile([P, NCT, TQ], FP8, tag="xcl")
        src0h = xh[:, 0:TQ].rearrange("(c p) t -> p c t", p=P)
        src0l = xl[:, 0:TQ].rearrange("(c p) t -> p c t", p=P)
        nc.sync.dma_start(x0h[:, 0:4, :], src0h[:, 0:4, :])
        nc.sync.dma_start(x0h[:, 4:8, :], src0h[:, 4:8, :])
        nc.sync.dma_start(bqk_sb, bqk[:])
        nc.sync.dma_start(wqk_sb[:, :, 0:512], wqk8[:, :, 0:512])
        nc.sync.dma_start(tri_sb, tri[:])
        nc.sync.dma_start(x0l, src0l)
        nc.sync.dma_start(wvh_sb, wvh[:])
        nc.sync.dma_start(wqk_sb[:, :, 512:1024], wqk8[:, :, 512:1024])
        nc.sync.dma_start(wvl_sb, wvl[:])
        nc.sync.dma_start(wvh4_sb, wvh4[:])
        nc.sync.dma_start(bv_sb[0:1, :], bv[:])
        nc.gpsimd.partition_broadcast(bv_sb[:, :], bv_sb[0:1, :])
        bv_hd = bv_sb.rearrange("p (h d) -> p h d", h=NHL)
        # ones columns of the augmented V carry the 64x v scale
        nc.vector.memset(v_sb[:, :, :, HD : HD + 1], VSC)

        s1_compute(0, x0h, x0l)
        nc.sync.dma_start(wp_sb, wp[:])
        # stage-3 back-fill: older tokens' projections run inside the later
        # (ACT-bound) phases; the last phase's own tiles go at the end
        S3_FILL = {2: (0, 1, 2, 3), 3: (4, 5, 6, 7, 8, 9, 10, 11)}
        units0 = None
        for t in range(NIT):
            fills = []
            if t + 1 < NIT:
                xih, xil = s1_load(t + 1)
                for mi in range(4):
                    fills.append(
                        lambda t1=t + 1, x_=xih, mi_=mi: s1_qk_block(t1, x_, mi_)
                    )
                for s in range(TQ // P):
                    fills.append(
                        lambda t1=t + 1, a_=xih, b_=xil, s_=s: s1_v_block(
                            t1, a_, b_, s_
                        )
                    )
                for mi in range(4, 8):
                    fills.append(
                        lambda t1=t + 1, x_=xih, mi_=mi: s1_qk_block(t1, x_, mi_)
                    )
            for tt in S3_FILL.get(t, ()):
                fills.append(lambda tt_=tt: s3_tile(tt_))
            units0 = att_phase(
                t, fills, init_units=units0, hoist_next=(t + 1 < NIT)
            )
    nc.compile()
    return nc


_NC_CACHE = None


def _get_nc():
    global _NC_CACHE
    if _NC_CACHE is None:
        _NC_CACHE = build_kernel()
    return _NC_CACHE


def _fp8(a):
    return np.asarray(a, np.float32).astype(ml_dtypes.float8_e4m3)


def _shard_inputs(x, w_qkv, b_qkv, w_proj):
    """Build the 8 per-core input maps. Core id = 2*batch + head_group."""
    bf = ml_dtypes.bfloat16
    tri01 = np.where(
        np.arange(P)[None, :] >= np.arange(P)[:, None], 1.0, 0.0
    )
    tri_np = np.ascontiguousarray(
        np.stack([tri01, tri01], axis=1)
    ).astype(bf)  # [P, 2, P]

    # q/k m-block row permutation: m in 0..7 -> (q|k, hg=(m%4)//2, pl=m%2);
    # partition p = 32*gg + i  ->  local head 4*hg + gg, dim 32*pl + i
    perm = np.empty((8, P), np.int64)
    for m in range(8):
        base = 0 if m < 4 else 1024
        mm = m % 4
        hg, pl = mm // 2, mm % 2
        for gg in range(4):
            h = 4 * hg + gg
            perm[m, 32 * gg : 32 * gg + 32] = (
                base + 64 * h + 32 * pl + np.arange(32)
            )

    def pack_w(rows_w, width):
        # [width f, 1024 c] -> [P part(c%128), NCT, width]
        return np.ascontiguousarray(
            rows_w.T.reshape(NCT, P, width).transpose(1, 0, 2)
        )

    in_maps = []
    for b in range(B):
        xt = np.ascontiguousarray(x[b].T)          # [C, T] f32
        xh_q = _fp8(xt * 0.25)                     # hi: x/4
        xl_q = _fp8((xt - xh_q.astype(np.float32) * 4.0) * 16.0)  # lo: 16*dx
        for g in range(2):
            s = slice(g * 512, (g + 1) * 512)
            rows = perm + g * 512
            wqk_full = w_qkv[rows[M_ORDER].reshape(-1)]  # [1024 f, 1024 c]
            wqk8_arr = _fp8(pack_w(wqk_full, 1024) * 256.0)
            bqk_full = b_qkv[rows.reshape(-1)] * VSC  # indexed by m, not mi
            bqk_arr = np.ascontiguousarray(bqk_full.reshape(8, P).T)
            wv_rows = w_qkv[2048:3072][s]          # [512 f, 1024 c]
            wvh_arr = _fp8(pack_w(wv_rows, 512) * 256.0)
            dv = pack_w(wv_rows, 512) - wvh_arr.astype(np.float32) / 256.0
            wvl_arr = _fp8(dv * 256.0)
            wvh4_arr = _fp8(pack_w(wv_rows, 512) * 4.0)
            bv_arr = np.ascontiguousarray(
                b_qkv[2048:3072][s][None, :] * VSC
            )
            wp_rhs = w_proj[:, s].T                # [512 hd, 1024 o]
            wp_arr = np.ascontiguousarray(
                wp_rhs.reshape(NPAIR, P, 1024).transpose(1, 0, 2)
            ).astype(bf)
            in_maps.append(
                {
                    "xh": xh_q,
                    "xl": xl_q,
                    "wqk8": wqk8_arr,
                    "bqk": bqk_arr.astype(np.float32),
                    "wvh": wvh_arr,
                    "wvl": wvl_arr,
                    "wvh4": wvh4_arr,
                    "bv": bv_arr.astype(np.float32),
                    "wp": wp_arr,
                    "tri": tri_np,
                }
            )
    return in_maps


def kernel(x, w_qkv, b_qkv, w_proj, b_proj, _trace=False, _trace_kwargs=None):
    x = np.asarray(x, dtype=np.float32)
    w_qkv = np.asarray(w_qkv, dtype=np.float32)
    b_qkv = np.asarray(b_qkv, dtype=np.float32)
    w_proj = np.asarray(w_proj, dtype=np.float32)
    b_proj = np.asarray(b_proj, dtype=np.float32)

    nc = _get_nc()
    in_maps = _shard_inputs(x, w_qkv, b_qkv, w_proj)
    res = run_bass_kernel_spmd(
        nc, in_maps, core_ids=list(range(8)), trace=_trace,
        **(_trace_kwargs or {}),
    )
    out = np.empty((B, T, C), np.float32)
    for b in range(B):
        out[b] = res.results[2 * b]["out"] + res.results[2 * b + 1]["out"] + b_proj
    if _trace:
        return out, res
    return out
